# revision 39
# baseline (speedup 1.0000x reference)
"""Trainium2 Bass kernel for nn_CrossAttention_40802189312391.

Sharding: 8 cores = (batch b in {0,1}) x (head-group g in {0..3}, 4 heads each).
Algebraic fusion: k1 = y @ (w_k @ w_q).T, v1 = y @ (w_v @ w_q).T — the yq
intermediate is never materialized. Each core computes a partial projection
output [2048, 1024] (contraction over its 256 head dims); host sums the 4
group partials per batch and adds b_proj.

Device program (per core). The ACT engine (exp over all 16.8M scores,
~133us) is the roofline; everything else is scheduled to hide under it:
  preA:     qs0/qs1 (qT), ks0/ks1 (k1T f32r, v1 bf16) through the scores
            psum banks before attention starts.
  qc loop:  per key-tile kt: row-packed f32r score matmuls into pssh[0/1]
            [128,1024] (2 heads each), exp (scale folded, bf16 out) to a
            P tile [128,2048]; PV with stationary=P chunk [keys,128q] and
            moving=v1 bf16 [keys,64] into psum_o [q,64] regions (full
            128-wide stationary vs 65 in the naive orientation), plus a
            1-col matmul per (head,qsub) accumulating the softmax
            denominator. Background work (remaining KV slices, later qT,
            previous qc's normalize/transpose/proj) is interleaved into
            the per-kt PE slack so ACT never starves.
  post(qc): per-q reciprocal of the denominator column, normalization
            fused into the psum->SBUF copy (DVE/GpSimd tensor_scalar_mul,
            bf16), PE transpose (identity matmul) back to [dims,q] for
            the projection, then out = outT.T @ w_proj streamed to DRAM.
"""
import sys
sys.path.insert(0, "/opt/trn_rl_repo")
import numpy as np

import concourse.bass as bass
import concourse.mybir as mybir
import concourse.tile as tile
from concourse import bacc

F32 = mybir.dt.float32
F32R = mybir.dt.float32r
BF16 = mybir.dt.bfloat16

DIM = 1024
Q_DIM = 768
HEADS = 16
HD = 64
SCALE = HD ** -0.5
B, N, N1 = 2, 2048, 2048
NCORES = 8
GDIM = 256          # head dims per core (4 heads)


def build_nc(nq=N, nk=N1, repeat=1):
    """Build the SPMD program. nq/nk scaled down for simulator runs.
    repeat>1 replicates the whole compute for timing (T(R)-T(1))/(R-1)."""
    QC = nq // 512       # q chunks
    KT = nk // 128       # key tiles
    KS = nk // 512       # key slices in KV phase
    XC = DIM // 128      # x feature chunks (8)
    YC = Q_DIM // 128    # y feature chunks (6)

    nc = bacc.Bacc("TRN2", target_bir_lowering=False, debug=False,
                   num_devices=NCORES)

    xT = nc.dram_tensor("xT", [DIM, nq], F32R, kind="ExternalInput")
    yT = nc.dram_tensor("yT", [Q_DIM, nk], F32R, kind="ExternalInput")
    wqT = nc.dram_tensor("wqT", [DIM, GDIM], F32R, kind="ExternalInput")
    wkT = nc.dram_tensor("wkT", [Q_DIM, GDIM], F32R, kind="ExternalInput")
    wvT = nc.dram_tensor("wvT", [Q_DIM, GDIM], F32R, kind="ExternalInput")
    wpT = nc.dram_tensor("wpT", [GDIM, DIM], F32R, kind="ExternalInput")
    identb = nc.dram_tensor("identb", [128, 128], F32, kind="ExternalInput")
    onesb = nc.dram_tensor("onesb", [128, 4], BF16, kind="ExternalInput")
    out = nc.dram_tensor("out", [nq, DIM], F32, kind="ExternalOutput")

    EXP = mybir.ActivationFunctionType.Exp

    with tile.TileContext(nc) as tc:
        with tc.tile_pool(name="weights", bufs=1) as wpool, \
             tc.tile_pool(name="persist", bufs=1) as pe, \
             tc.tile_pool(name="stream", bufs=2) as stream, \
             tc.tile_pool(name="work", bufs=3) as work:

            # --- weight tiles (DMAs emitted just-in-time below) ---
            # wq split in two tiles so the first q matmuls start after
            # half the wq bytes have landed
            wq_sb2 = [wpool.tile([128, XC // 2 * GDIM], F32R,
                                 name=f"wq_sb{i}") for i in range(2)]
            wk_sb = wpool.tile([128, YC * GDIM], F32R, name="wk_sb")
            wv_sb = wpool.tile([128, YC * GDIM], F32R, name="wv_sb")
            wp_sb = wpool.tile([128, 2 * DIM], F32R, name="wp_sb")
            ident_sb = wpool.tile([128, 128], F32, name="ident_sb")
            ones_sb = wpool.tile([128, 4], BF16, name="ones_sb")

            # single strided DMA per weight: [chunk*128 + p, c] DRAM rows ->
            # SBUF partition p, cols chunk*W + c
            def dma_w(sb, dram, nchunk, w):
                nc.sync.dma_start(
                    sb.rearrange("p (k c) -> p k c", k=nchunk),
                    dram.rearrange("(k p) c -> p k c", k=nchunk))

            def dma_wq():
                h = XC // 2
                dma_w(wq_sb2[0], wqT[0:h * 128, :], h, GDIM)
                dma_w(wq_sb2[1], wqT[h * 128:XC * 128, :], h, GDIM)

            def dma_wk():
                dma_w(wk_sb, wkT, YC, GDIM)

            def dma_wv():
                dma_w(wv_sb, wvT, YC, GDIM)

            def dma_rest():
                nc.sync.dma_start(ones_sb[:, :], onesb[:, :])
                nc.sync.dma_start(ident_sb[:, :], identb[:, :])
                dma_w(wp_sb, wpT, 2, DIM)

            # --- persistent activations ---
            k1T_sb = [pe.tile([128, nk], F32R, name=f"k1T_{p}",
                              tag=f"k1T_{p}") for p in range(2)]
            qT_sb = [pe.tile([128, nq], F32R, name=f"qT_{p}",
                             tag=f"qT_{p}") for p in range(2)]
            v1_sb = [pe.tile([128, 256], BF16, name=f"v1_{kt}",
                             tag=f"v1_{kt}") for kt in range(KT)]
            outT_sb = [pe.tile([128, nq], F32R, name=f"outT_{p}",
                               tag=f"outT_{p}") for p in range(2)]

            # preload the Exp table off the critical path
            z0 = work.tile([128, 8], F32, name="z0", tag="z0", bufs=1)
            z1 = work.tile([128, 8], F32, name="z1", tag="z1", bufs=1)
            nc.vector.memset(z0[:], 0.0)
            nc.scalar.activation(z1[:], z0[:], EXP, scale=1.0)

            # --- streamed input tiles (cached per slice) ---
            xtiles_c, ytiles_c = {}, {}

            def _stream_pair(cache, key, src, nchunk, tag):
                # two tiles per slice (one DMA each) so compute starts
                # after the first half lands
                if key not in cache:
                    h = nchunk // 2
                    sls = []
                    for i, n in ((0, h), (1, nchunk - h)):
                        t = stream.tile([128, n * 512], F32R, name=tag,
                                        tag=f"{tag}{i}", bufs=2)
                        nc.sync.dma_start(
                            t.rearrange("p (k c) -> p k c", k=n),
                            src[i * h * 128:(i * h + n) * 128, :]
                            .rearrange("(k p) c -> p k c", k=n))
                        sls += [t[:, c * 512:(c + 1) * 512] for c in range(n)]
                    cache[key] = sls
                return cache[key]

            def xtiles(qs):
                return _stream_pair(xtiles_c, qs,
                                    xT[:, qs * 512:(qs + 1) * 512], XC, "xt")

            def ytiles(ks):
                return _stream_pair(ytiles_c, ks,
                                    yT[:, ks * 512:(ks + 1) * 512], YC, "yt")

            # --- building-block groups (each ends with a psum evacuation) --
            def q_group(qs, p, ps):
                xts = xtiles(qs)
                h = XC // 2
                for kc in range(XC):
                    wq = wq_sb2[kc // h]
                    kk = kc % h
                    nc.tensor.matmul(
                        ps,
                        wq[:, kk * GDIM + p * 128:
                           kk * GDIM + (p + 1) * 128],
                        xts[kc], start=(kc == 0), stop=(kc == XC - 1))
                nc.vector.tensor_copy(
                    qT_sb[p][:, qs * 512:(qs + 1) * 512], ps)

            def k_group(ks, p, ps):
                yts = ytiles(ks)
                for ycc in range(YC):
                    nc.tensor.matmul(
                        ps,
                        wk_sb[:, ycc * GDIM + p * 128:
                              ycc * GDIM + (p + 1) * 128],
                        yts[ycc], start=(ycc == 0), stop=(ycc == YC - 1))
                nc.vector.tensor_copy(
                    k1T_sb[p][:, ks * 512:(ks + 1) * 512], ps)

            def v_group(ks, j, ps):
                yts = ytiles(ks)
                for ycc in range(YC):
                    nc.tensor.matmul(
                        ps,
                        yts[ycc][:, j * 128:(j + 1) * 128],
                        wv_sb[:, ycc * GDIM:(ycc + 1) * GDIM],
                        start=(ycc == 0), stop=(ycc == YC - 1))
                nc.vector.tensor_copy(v1_sb[ks * 4 + j][:], ps)

            for _rep in range(repeat):
              xtiles_c.clear()
              ytiles_c.clear()
              with tc.tile_pool(name="attn1", bufs=1, space="PSUM") as aps:
                pssh = [aps.tile([128, 1024], F32, name=f"pssh{u}",
                                 tag=f"pssh{u}") for u in range(2)]
                psum_o = [aps.tile([128, 512], F32, name=f"pso{c}",
                                   tag=f"pso{c}") for c in range(2)]
                # den regions in cols 0..127; cols 128..383 double as the
                # two transpose output regions (same dtype, same bank)
                den_ps = aps.tile([128, 512], F32, name="den", tag="den")
                trans_ps = [den_ps[:, 128 + u * 128:256 + u * 128]
                            for u in range(2)]

                # ---- preA: qs0/qs1, ks0 through the pssh banks, with the
                # weight DMAs emitted just before their first consumer.
                # 4-way region rotation so WAR tile waits land 3 groups back.
                regions = [pssh[0][:, 0:512], pssh[1][:, 0:512],
                           pssh[0][:, 512:1024], pssh[1][:, 512:1024]]
                rr = [0]

                def next_region(cols=512):
                    r = regions[rr[0] % 4]
                    rr[0] += 1
                    return r if cols == 512 else r[:, 0:cols]

                dma_wq()
                q_group(0, 0, next_region())
                q_group(0, 1, next_region())
                dma_wk()
                k_group(0, 0, next_region())
                k_group(0, 1, next_region())
                dma_wv()
                for j in range(4):
                    v_group(0, j, next_region(256))
                if QC > 1:
                    q_group(1, 0, next_region())
                    q_group(1, 1, next_region())
                if KS > 1:
                    ytiles(1)          # prefetch ahead of the ks1 thunks
                dma_rest()

                # ---- background thunks interleaved into the qc loops ----
                # pre[kt] thunks run BEFORE that kt's body (the previous
                # qc's psum_o/den_ps readout must precede this qc's PV
                # resets — PV emission lags 4 kts to give it room);
                # post[kt] thunks run after the body.
                def bg_schedule(qc, kv2_tiles, attn2_tiles):
                    pre, post = {}, {}

                    def add(d, kt, fn):
                        d.setdefault(kt, []).append(fn)

                    if qc == 0 and KS > 1:
                        kvk, kvv = kv2_tiles
                        for i, ks in enumerate(range(1, KS)):
                            base = 0 if i == 0 else 3 + (i - 1) * 6
                            ksl = [0, 0] if i == 0 else [base, base + 1]
                            vsl = ([1, 1, 2, 2] if i == 0
                                   else [base + 2 + j for j in range(4)])
                            add(pre, max(0, ksl[0] - 2),
                                lambda ks=ks: ytiles(ks) and None)
                            for p in range(2):
                                add(post, ksl[p], (lambda ks=ks, p=p:
                                                   k_group(ks, p, kvk)))
                            for j in range(4):
                                add(post, vsl[j], (lambda ks=ks, j=j:
                                                   v_group(ks, j, kvv)))
                    if qc >= 1:
                        trans_ps, psp = attn2_tiles
                        for qs4 in range(4):
                            add(pre, 1 + qs4, (lambda qs4=qs4:
                                               post_chunk(qc - 1, qs4,
                                                          trans_ps)))
                        for nt in range(4):
                            for dc in range(2):
                                eng = nc.vector if dc == 0 else nc.gpsimd
                                add(post, 4 + nt * 2 + dc,
                                    (lambda nt=nt, dc=dc, eng=eng:
                                     proj_group(qc - 1, nt, dc, psp[:], eng)))
                        # remaining qT chunks, latest-needed last
                        qs = qc + 1
                        if qs < QC:
                            for p in range(2):
                                add(post, 12 + 2 * p, (lambda qs=qs, p=p:
                                                       q_group(qs, p, psp[:])))
                    return pre, post

                def post_chunk(qc, qs4, trans_ps):
                    # normalize psum_o[q, dims] by the denominator column,
                    # transpose to [dims, q] into outT for the projection.
                    asb = {}
                    for cc in range(2):
                        asb[cc] = work.tile([128, 128], F32, name="asb",
                                            tag=f"asb{cc}", bufs=2)
                    for h in range(4):
                        cc, e = h // 2, h % 2
                        idx = h * 4 + qs4
                        rec = work.tile([128, 1], F32, name="rec",
                                        tag=f"rec{h}", bufs=2)
                        nc.vector.reciprocal(
                            rec[:], den_ps[:, idx * 8:idx * 8 + 1])
                        eng = nc.vector if e == 0 else nc.gpsimd
                        eng.tensor_scalar_mul(
                            asb[cc][:, e * 64:(e + 1) * 64],
                            psum_o[cc][:, e * 256 + qs4 * 64:
                                       e * 256 + (qs4 + 1) * 64],
                            rec[:, 0:1])
                    for cc in range(2):
                        tp = trans_ps[cc]
                        nc.tensor.transpose(tp, asb[cc][:], ident_sb[:])
                        eng = nc.vector if cc == 0 else nc.gpsimd
                        eng.tensor_copy(
                            outT_sb[cc][:, qc * 512 + qs4 * 128:
                                        qc * 512 + (qs4 + 1) * 128], tp)

                def proj_group(qc, nt, dc, ps, eng=None):
                    q0 = qc * 512 + nt * 128
                    for cc in range(2):
                        nc.tensor.matmul(
                            ps,
                            outT_sb[cc][:, q0:q0 + 128],
                            wp_sb[:, cc * DIM + dc * 512:
                                  cc * DIM + (dc + 1) * 512],
                            start=(cc == 0), stop=(cc == 1))
                    ot = work.tile([128, 512], F32, name="ot", tag="ot",
                                   bufs=3)
                    (eng or nc.vector).tensor_copy(ot[:], ps)
                    nc.sync.dma_start(
                        out[q0:q0 + 128, dc * 512:(dc + 1) * 512], ot[:])

                # ---- attention qc loop ----
                PV_LAG = 4

                def pv_emit(qc, kt, P, qs4s=(0, 1, 2, 3)):
                    for h in range(4):
                        cc, e = h // 2, h % 2
                        for qs4 in qs4s:
                            st = P[:, h * 512 + qs4 * 128:
                                   h * 512 + (qs4 + 1) * 128]
                            nc.tensor.matmul(
                                psum_o[cc][:, e * 256 + qs4 * 64:
                                           e * 256 + (qs4 + 1) * 64],
                                st, v1_sb[kt][:, h * 64:(h + 1) * 64],
                                start=(kt == 0), stop=(kt == KT - 1))
                            idx = h * 4 + qs4
                            nc.tensor.matmul(
                                den_ps[:, idx * 8:idx * 8 + 1],
                                st, ones_sb[:, 0:1],
                                start=(kt == 0), stop=(kt == KT - 1))

                def qc_loop(qc, kv2_tiles, attn2_tiles, flush=True):
                    pre, post = bg_schedule(qc, kv2_tiles, attn2_tiles)
                    Ps = {}
                    for kt in range(KT):
                        for fn in pre.get(kt, ()):
                            fn()
                        P = work.tile([128, 2048], BF16, name="P", tag="P",
                                      bufs=PV_LAG + 3)
                        Ps[kt] = P
                        for half in range(2):
                            for i, h in enumerate((2 * half, 2 * half + 1)):
                                p, e = h // 2, h % 2
                                nc.tensor.matmul(
                                    pssh[half][:, i * 512:(i + 1) * 512],
                                    k1T_sb[p][e * 64:(e + 1) * 64,
                                              kt * 128:(kt + 1) * 128],
                                    qT_sb[p][e * 64:(e + 1) * 64,
                                             qc * 512:(qc + 1) * 512],
                                    start=True, stop=True,
                                    tile_position=(e * 64, 0))
                            nc.scalar.activation(
                                P[:, half * 1024:(half + 1) * 1024],
                                pssh[half][:], EXP, scale=SCALE)
                        if kt >= PV_LAG:
                            pv_emit(qc, kt - PV_LAG, Ps.pop(kt - PV_LAG))
                        for fn in post.get(kt, ()):
                            fn()
                    if flush:
                        for kt in range(max(0, KT - PV_LAG), KT):
                            pv_emit(qc, kt, Ps.pop(kt))
                    return Ps

                # qc0 with the kv2 pool (remaining KV slices in background)
                with tc.tile_pool(name="kv2", bufs=1, space="PSUM") as kps:
                    kvk = kps.tile([128, 512], F32, name="kvk", tag="kvk")
                    qc_loop(0, (kvk[:, :], kvk[:, 0:256]), None)

                with tc.tile_pool(name="attn2", bufs=1, space="PSUM") as a2:
                    psp = a2.tile([128, 512], F32, name="psp", tag="psp")
                    Ps = {}
                    for qc in range(1, QC):
                        Ps = qc_loop(qc, None, (trans_ps, psp),
                                     flush=(qc != QC - 1))
                    if QC == 1:
                        Ps = {}
                    # tail: last qc's PV flush + normalize + projection,
                    # per q-subtile so the chains overlap the flush. The
                    # pssh banks are free after the last exp — rotate the
                    # proj psum across 5 banks and split copies across
                    # engines so nothing serializes.
                    tail_ps = [psp[:, :],
                               pssh[0][:, 0:512], pssh[0][:, 512:1024],
                               pssh[1][:, 0:512], pssh[1][:, 512:1024]]
                    ti = 0
                    fl = range(max(0, KT - PV_LAG), KT) if QC > 1 else []
                    for qs4 in range(4):
                        for kt in fl:
                            pv_emit(QC - 1, kt, Ps[kt], qs4s=(qs4,))
                        post_chunk(QC - 1, qs4, trans_ps)
                        for dc in range(2):
                            eng = nc.vector if dc == 0 else nc.gpsimd
                            proj_group(QC - 1, qs4, dc,
                                       tail_ps[ti % 5], eng)
                            ti += 1

    nc.compile()
    return nc


# ---------------- host-side runner (inlined, self-contained) ----------------
class _Runner:
    def __init__(self, nc, n_cores):
        import jax
        from jax.sharding import Mesh, PartitionSpec
        from jax.experimental.shard_map import shard_map
        from concourse.bass2jax import (
            _bass_exec_p, install_neuronx_cc_hook, partition_id_tensor)
        install_neuronx_cc_hook()
        self.jax = jax
        self.n_cores = n_cores
        partition_name = (nc.partition_id_tensor.name
                          if nc.partition_id_tensor else None)
        in_names, out_names, out_avals, zero_outs = [], [], [], []
        for alloc in nc.m.functions[0].allocations:
            if not isinstance(alloc, mybir.MemoryLocationSet):
                continue
            name = alloc.memorylocations[0].name
            if alloc.kind == "ExternalInput":
                if name != partition_name:
                    in_names.append(name)
            elif alloc.kind == "ExternalOutput":
                shape = tuple(alloc.tensor_shape)
                dtype = mybir.dt.np(alloc.dtype)
                out_names.append(name)
                out_avals.append(jax.core.ShapedArray(shape, dtype))
                zero_outs.append(np.zeros(shape, dtype))
        self.in_names, self.out_names = in_names, out_names
        self.out_avals, self.zero_outs = out_avals, zero_outs
        n_params = len(in_names)
        self.n_params = n_params
        all_in = list(in_names) + list(out_names)
        if partition_name is not None:
            all_in.append(partition_name)

        def _body(*args):
            operands = list(args)
            if partition_name is not None:
                operands.append(partition_id_tensor())
            return tuple(_bass_exec_p.bind(
                *operands, out_avals=tuple(out_avals),
                in_names=tuple(all_in), out_names=tuple(out_names),
                lowering_input_output_aliases=(),
                sim_require_finite=True, sim_require_nnan=True, nc=nc))

        devices = jax.devices()[:n_cores]
        self.mesh = Mesh(np.asarray(devices), ("core",))
        n_outs = len(out_names)
        self.fn = jax.jit(
            shard_map(_body, mesh=self.mesh,
                      in_specs=(PartitionSpec("core"),) * (n_params + n_outs),
                      out_specs=(PartitionSpec("core"),) * n_outs,
                      check_rep=False),
            keep_unused=True)
        self._dev_args = None

    def stage_inputs(self, in_maps):
        from jax.sharding import NamedSharding, PartitionSpec
        per_core = [[np.asarray(m[n]) for n in self.in_names] for m in in_maps]
        concat_in = [
            np.concatenate([per_core[c][i] for c in range(self.n_cores)],
                           axis=0) for i in range(self.n_params)]
        concat_zeros = [np.zeros((self.n_cores * z.shape[0], *z.shape[1:]),
                                 z.dtype) for z in self.zero_outs]
        sharding = NamedSharding(self.mesh, PartitionSpec("core"))
        self._dev_args = [self.jax.device_put(a, sharding)
                          for a in (*concat_in, *concat_zeros)]
        self.jax.block_until_ready(self._dev_args)

    def run(self):
        o = self.fn(*self._dev_args)
        self.jax.block_until_ready(o)
        return o

    def results(self, out_arrs):
        return [
            {n: np.asarray(out_arrs[i]).reshape(
                self.n_cores, *self.out_avals[i].shape)[c]
             for i, n in enumerate(self.out_names)}
            for c in range(self.n_cores)]


def make_in_maps(x, y, w_q, w_qkv, w_proj):
    x = np.asarray(x, np.float32)
    y = np.asarray(y, np.float32)
    w_q = np.asarray(w_q, np.float32)
    w_qkv = np.asarray(w_qkv, np.float32)
    w_proj = np.asarray(w_proj, np.float32)
    w_qq = w_qkv[0:DIM]
    wk_f = w_qkv[DIM:2 * DIM] @ w_q      # [1024, 768]
    wv_f = w_qkv[2 * DIM:3 * DIM] @ w_q
    bf16 = mybir.dt.np(BF16)
    ident = np.eye(128, dtype=np.float32)
    ones = np.ones((128, 4), bf16)
    in_maps = []
    for core in range(NCORES):
        b, g = core // 4, core % 4
        hs = slice(g * GDIM, (g + 1) * GDIM)
        in_maps.append({
            "xT": np.ascontiguousarray(x[b].T),
            "yT": np.ascontiguousarray(y[b].T),
            "wqT": np.ascontiguousarray(w_qq[hs].T),
            "wkT": np.ascontiguousarray(wk_f[hs].T),
            "wvT": np.ascontiguousarray(wv_f[hs].T),
            "wpT": np.ascontiguousarray(w_proj[:, hs].T),
            "identb": ident,
            "onesb": ones,
        })
    return in_maps


_RUNNER = None


def kernel(x, y, w_q, w_qkv, w_proj, b_proj):
    global _RUNNER
    in_maps = make_in_maps(x, y, w_q, w_qkv, w_proj)
    if _RUNNER is None:
        _RUNNER = _Runner(build_nc(), NCORES)
    _RUNNER.stage_inputs(in_maps)
    res = _RUNNER.results(_RUNNER.run())
    full = np.zeros((B, N, DIM), np.float32)
    for core in range(NCORES):
        full[core // 4] += res[core]["out"]
    full += np.asarray(b_proj, np.float32)
    return full


# revision 40
# speedup vs baseline: 1.0202x; 1.0202x over previous
"""Trainium2 Bass kernel for nn_CrossAttention_40802189312391.

Sharding: 8 cores = (batch b in {0,1}) x (head-group g in {0..3}, 4 heads each).
Algebraic fusion: k1 = y @ (w_k @ w_q).T, v1 = y @ (w_v @ w_q).T — the yq
intermediate is never materialized. Each core computes a partial projection
output [2048, 1024] (contraction over its 256 head dims); host sums the 4
group partials per batch and adds b_proj.

Device program (per core). The ACT engine (exp over all 16.8M scores,
~133us) is the roofline; everything else is scheduled to hide under it:
  preA:     qs0/qs1 (qT), ks0/ks1 (k1T f32r, v1 bf16) through the scores
            psum banks before attention starts.
  qc loop:  per key-tile kt: row-packed f32r score matmuls into pssh[0/1]
            [128,1024] (2 heads each), exp (scale folded, bf16 out) to a
            P tile [128,2048]; PV with stationary=P chunk [keys,128q] and
            moving=v1 bf16 [keys,64] into psum_o [q,64] regions (full
            128-wide stationary vs 65 in the naive orientation), plus a
            1-col matmul per (head,qsub) accumulating the softmax
            denominator. Background work (remaining KV slices, later qT,
            previous qc's normalize/transpose/proj) is interleaved into
            the per-kt PE slack so ACT never starves.
  post(qc): per-q reciprocal of the denominator column, normalization
            fused into the psum->SBUF copy (DVE/GpSimd tensor_scalar_mul,
            bf16), PE transpose (identity matmul) back to [dims,q] for
            the projection, then out = outT.T @ w_proj streamed to DRAM.
"""
import sys
sys.path.insert(0, "/opt/trn_rl_repo")
import numpy as np

import concourse.bass as bass
import concourse.mybir as mybir
import concourse.tile as tile
from concourse import bacc

F32 = mybir.dt.float32
F32R = mybir.dt.float32r
BF16 = mybir.dt.bfloat16

DIM = 1024
Q_DIM = 768
HEADS = 16
HD = 64
SCALE = HD ** -0.5
B, N, N1 = 2, 2048, 2048
NCORES = 8
GDIM = 256          # head dims per core (4 heads)


def build_nc(nq=N, nk=N1, repeat=1):
    """Build the SPMD program. nq/nk scaled down for simulator runs.
    repeat>1 replicates the whole compute for timing (T(R)-T(1))/(R-1)."""
    QC = nq // 512       # q chunks
    KT = nk // 128       # key tiles
    KS = nk // 512       # key slices in KV phase
    XC = DIM // 128      # x feature chunks (8)
    YC = Q_DIM // 128    # y feature chunks (6)

    nc = bacc.Bacc("TRN2", target_bir_lowering=False, debug=False,
                   num_devices=NCORES)

    xT = nc.dram_tensor("xT", [DIM, nq], F32R, kind="ExternalInput")
    yT = nc.dram_tensor("yT", [Q_DIM, nk], F32R, kind="ExternalInput")
    wqT = nc.dram_tensor("wqT", [DIM, GDIM], F32R, kind="ExternalInput")
    wkT = nc.dram_tensor("wkT", [Q_DIM, GDIM], F32R, kind="ExternalInput")
    wvT = nc.dram_tensor("wvT", [Q_DIM, GDIM], F32R, kind="ExternalInput")
    wpT = nc.dram_tensor("wpT", [GDIM, DIM], F32R, kind="ExternalInput")
    identb = nc.dram_tensor("identb", [128, 128], F32, kind="ExternalInput")
    onesb = nc.dram_tensor("onesb", [128, 4], BF16, kind="ExternalInput")
    out = nc.dram_tensor("out", [nq, DIM], F32, kind="ExternalOutput")

    EXP = mybir.ActivationFunctionType.Exp

    with tile.TileContext(nc) as tc:
        with tc.tile_pool(name="weights", bufs=1) as wpool, \
             tc.tile_pool(name="persist", bufs=1) as pe, \
             tc.tile_pool(name="stream", bufs=2) as stream, \
             tc.tile_pool(name="work", bufs=3) as work:

            # --- weight tiles (DMAs emitted just-in-time below) ---
            # wq split in two tiles so the first q matmuls start after
            # half the wq bytes have landed
            wq_sb2 = [wpool.tile([128, XC // 2 * GDIM], F32R,
                                 name=f"wq_sb{i}") for i in range(2)]
            wk_sb = wpool.tile([128, YC * GDIM], F32R, name="wk_sb")
            wv_sb = wpool.tile([128, YC * GDIM], F32R, name="wv_sb")
            wp_sb = wpool.tile([128, 2 * DIM], F32R, name="wp_sb")
            ident_sb = wpool.tile([128, 128], F32, name="ident_sb")
            ones_sb = wpool.tile([128, 4], BF16, name="ones_sb")

            # single strided DMA per weight: [chunk*128 + p, c] DRAM rows ->
            # SBUF partition p, cols chunk*W + c
            def dma_w(sb, dram, nchunk, w):
                nc.sync.dma_start(
                    sb.rearrange("p (k c) -> p k c", k=nchunk),
                    dram.rearrange("(k p) c -> p k c", k=nchunk))

            def dma_wq():
                h = XC // 2
                dma_w(wq_sb2[0], wqT[0:h * 128, :], h, GDIM)
                dma_w(wq_sb2[1], wqT[h * 128:XC * 128, :], h, GDIM)

            def dma_wk():
                dma_w(wk_sb, wkT, YC, GDIM)

            def dma_wv():
                dma_w(wv_sb, wvT, YC, GDIM)

            def dma_rest():
                nc.sync.dma_start(ones_sb[:, :], onesb[:, :])
                nc.sync.dma_start(ident_sb[:, :], identb[:, :])
                dma_w(wp_sb, wpT, 2, DIM)

            # --- persistent activations ---
            k1T_sb = [pe.tile([128, nk], F32R, name=f"k1T_{p}",
                              tag=f"k1T_{p}") for p in range(2)]
            qT_sb = [pe.tile([128, nq], F32R, name=f"qT_{p}",
                             tag=f"qT_{p}") for p in range(2)]
            v1_sb = [pe.tile([128, 256], BF16, name=f"v1_{kt}",
                             tag=f"v1_{kt}") for kt in range(KT)]
            outT_sb = [pe.tile([128, nq], F32R, name=f"outT_{p}",
                               tag=f"outT_{p}") for p in range(2)]

            # preload the Exp table off the critical path
            z0 = work.tile([128, 8], F32, name="z0", tag="z0", bufs=1)
            z1 = work.tile([128, 8], F32, name="z1", tag="z1", bufs=1)
            nc.vector.memset(z0[:], 0.0)
            nc.scalar.activation(z1[:], z0[:], EXP, scale=1.0)

            # --- streamed input tiles (cached per slice) ---
            xtiles_c, ytiles_c = {}, {}

            def _stream_pair(cache, key, src, nchunk, tag):
                # two tiles per slice (one DMA each) so compute starts
                # after the first half lands
                if key not in cache:
                    h = nchunk // 2
                    sls = []
                    for i, n in ((0, h), (1, nchunk - h)):
                        t = stream.tile([128, n * 512], F32R, name=tag,
                                        tag=f"{tag}{i}", bufs=2)
                        nc.sync.dma_start(
                            t.rearrange("p (k c) -> p k c", k=n),
                            src[i * h * 128:(i * h + n) * 128, :]
                            .rearrange("(k p) c -> p k c", k=n))
                        sls += [t[:, c * 512:(c + 1) * 512] for c in range(n)]
                    cache[key] = sls
                return cache[key]

            def xtiles(qs):
                return _stream_pair(xtiles_c, qs,
                                    xT[:, qs * 512:(qs + 1) * 512], XC, "xt")

            def ytiles(ks):
                return _stream_pair(ytiles_c, ks,
                                    yT[:, ks * 512:(ks + 1) * 512], YC, "yt")

            # --- building-block groups (each ends with a psum evacuation) --
            def q_group(qs, p, ps):
                xts = xtiles(qs)
                h = XC // 2
                for kc in range(XC):
                    wq = wq_sb2[kc // h]
                    kk = kc % h
                    nc.tensor.matmul(
                        ps,
                        wq[:, kk * GDIM + p * 128:
                           kk * GDIM + (p + 1) * 128],
                        xts[kc], start=(kc == 0), stop=(kc == XC - 1))
                nc.vector.tensor_copy(
                    qT_sb[p][:, qs * 512:(qs + 1) * 512], ps)

            def k_group(ks, p, ps):
                yts = ytiles(ks)
                for ycc in range(YC):
                    nc.tensor.matmul(
                        ps,
                        wk_sb[:, ycc * GDIM + p * 128:
                              ycc * GDIM + (p + 1) * 128],
                        yts[ycc], start=(ycc == 0), stop=(ycc == YC - 1))
                nc.vector.tensor_copy(
                    k1T_sb[p][:, ks * 512:(ks + 1) * 512], ps)

            def v_group(ks, j, ps):
                yts = ytiles(ks)
                for ycc in range(YC):
                    nc.tensor.matmul(
                        ps,
                        yts[ycc][:, j * 128:(j + 1) * 128],
                        wv_sb[:, ycc * GDIM:(ycc + 1) * GDIM],
                        start=(ycc == 0), stop=(ycc == YC - 1))
                nc.vector.tensor_copy(v1_sb[ks * 4 + j][:], ps)

            for _rep in range(repeat):
              xtiles_c.clear()
              ytiles_c.clear()
              with tc.tile_pool(name="attn1", bufs=1, space="PSUM") as aps:
                pssh = [aps.tile([128, 1024], F32, name=f"pssh{u}",
                                 tag=f"pssh{u}") for u in range(2)]
                psum_o = [aps.tile([128, 512], F32, name=f"pso{c}",
                                   tag=f"pso{c}") for c in range(2)]
                # den regions in cols 0..127; cols 128..383 double as the
                # two transpose output regions (same dtype, same bank)
                den_ps = aps.tile([128, 512], F32, name="den", tag="den")
                trans_ps = [den_ps[:, 128 + u * 128:256 + u * 128]
                            for u in range(2)]

                # ---- preA: qs0/qs1, ks0 through the pssh banks, with the
                # weight DMAs emitted just before their first consumer.
                # 4-way region rotation so WAR tile waits land 3 groups back.
                regions = [pssh[0][:, 0:512], pssh[1][:, 0:512],
                           pssh[0][:, 512:1024], pssh[1][:, 512:1024]]
                rr = [0]

                def next_region(cols=512):
                    r = regions[rr[0] % 4]
                    rr[0] += 1
                    return r if cols == 512 else r[:, 0:cols]

                dma_wq()
                q_group(0, 0, next_region())
                q_group(0, 1, next_region())
                dma_wk()
                k_group(0, 0, next_region())
                k_group(0, 1, next_region())
                dma_wv()
                for j in range(4):
                    v_group(0, j, next_region(256))
                if QC > 1:
                    q_group(1, 0, next_region())
                    q_group(1, 1, next_region())
                if KS > 1:
                    ytiles(1)          # prefetch ahead of the ks1 thunks
                dma_rest()

                # ---- background thunks interleaved into the qc loops ----
                # pre[kt] thunks run BEFORE that kt's body (the previous
                # qc's psum_o/den_ps readout must precede this qc's PV
                # resets — PV emission lags 4 kts to give it room);
                # post[kt] thunks run after the body.
                def bg_schedule(qc, kv2_tiles, attn2_tiles):
                    pre, post = {}, {}

                    def add(d, kt, fn):
                        d.setdefault(kt, []).append(fn)

                    if qc == 0 and KS > 1:
                        kvk, kvv = kv2_tiles
                        for i, ks in enumerate(range(1, KS)):
                            base = 0 if i == 0 else 3 + (i - 1) * 6
                            ksl = [0, 0] if i == 0 else [base, base + 1]
                            vsl = ([1, 1, 2, 2] if i == 0
                                   else [base + 2 + j for j in range(4)])
                            add(pre, max(0, ksl[0] - 2),
                                lambda ks=ks: ytiles(ks) and None)
                            for p in range(2):
                                add(post, ksl[p], (lambda ks=ks, p=p:
                                                   k_group(ks, p, kvk)))
                            for j in range(4):
                                add(post, vsl[j], (lambda ks=ks, j=j:
                                                   v_group(ks, j, kvv)))
                    if qc >= 1:
                        trans_ps, psp = attn2_tiles
                        for qs4 in range(4):
                            add(pre, 1 + qs4, (lambda qs4=qs4:
                                               post_chunk(qc - 1, qs4,
                                                          trans_ps)))
                        for nt in range(4):
                            for dc in range(2):
                                add(post, 4 + nt * 2 + dc,
                                    (lambda nt=nt, dc=dc:
                                     proj_group(qc - 1, nt, dc, psp[:])))
                        # remaining qT chunks, latest-needed last
                        qs = qc + 1
                        if qs < QC:
                            for p in range(2):
                                add(post, 12 + 2 * p, (lambda qs=qs, p=p:
                                                       q_group(qs, p, psp[:])))
                    return pre, post

                def post_chunk(qc, qs4, trans_ps):
                    # normalize psum_o[q, dims] by the denominator column,
                    # transpose to [dims, q] into outT for the projection.
                    asb = {}
                    for cc in range(2):
                        asb[cc] = work.tile([128, 128], F32, name="asb",
                                            tag=f"asb{cc}", bufs=2)
                    for h in range(4):
                        cc, e = h // 2, h % 2
                        idx = h * 4 + qs4
                        rec = work.tile([128, 1], F32, name="rec",
                                        tag=f"rec{h}", bufs=2)
                        nc.vector.reciprocal(
                            rec[:], den_ps[:, idx * 8:idx * 8 + 1])
                        eng = nc.vector if e == 0 else nc.gpsimd
                        eng.tensor_scalar_mul(
                            asb[cc][:, e * 64:(e + 1) * 64],
                            psum_o[cc][:, e * 256 + qs4 * 64:
                                       e * 256 + (qs4 + 1) * 64],
                            rec[:, 0:1])
                    for cc in range(2):
                        tp = trans_ps[cc]
                        nc.tensor.transpose(tp, asb[cc][:], ident_sb[:])
                        eng = nc.vector if cc == 0 else nc.gpsimd
                        eng.tensor_copy(
                            outT_sb[cc][:, qc * 512 + qs4 * 128:
                                        qc * 512 + (qs4 + 1) * 128], tp)

                def proj_group(qc, nt, dc, ps, eng=None):
                    q0 = qc * 512 + nt * 128
                    for cc in range(2):
                        nc.tensor.matmul(
                            ps,
                            outT_sb[cc][:, q0:q0 + 128],
                            wp_sb[:, cc * DIM + dc * 512:
                                  cc * DIM + (dc + 1) * 512],
                            start=(cc == 0), stop=(cc == 1))
                    ot = work.tile([128, 512], F32, name="ot", tag="ot",
                                   bufs=3)
                    (eng or nc.vector).tensor_copy(ot[:], ps)
                    nc.sync.dma_start(
                        out[q0:q0 + 128, dc * 512:(dc + 1) * 512], ot[:])

                # ---- attention qc loop ----
                PV_LAG = 4

                def pv_emit(qc, kt, P, qs4s=(0, 1, 2, 3)):
                    for h in range(4):
                        cc, e = h // 2, h % 2
                        for qs4 in qs4s:
                            st = P[:, h * 512 + qs4 * 128:
                                   h * 512 + (qs4 + 1) * 128]
                            nc.tensor.matmul(
                                psum_o[cc][:, e * 256 + qs4 * 64:
                                           e * 256 + (qs4 + 1) * 64],
                                st, v1_sb[kt][:, h * 64:(h + 1) * 64],
                                start=(kt == 0), stop=(kt == KT - 1))
                            idx = h * 4 + qs4
                            nc.tensor.matmul(
                                den_ps[:, idx * 8:idx * 8 + 1],
                                st, ones_sb[:, 0:1],
                                start=(kt == 0), stop=(kt == KT - 1))

                def qc_loop(qc, kv2_tiles, attn2_tiles, flush=True):
                    pre, post = bg_schedule(qc, kv2_tiles, attn2_tiles)
                    Ps = {}
                    for kt in range(KT):
                        for fn in pre.get(kt, ()):
                            fn()
                        P = work.tile([128, 2048], BF16, name="P", tag="P",
                                      bufs=PV_LAG + 3)
                        Ps[kt] = P
                        for half in range(2):
                            for i, h in enumerate((2 * half, 2 * half + 1)):
                                p, e = h // 2, h % 2
                                nc.tensor.matmul(
                                    pssh[half][:, i * 512:(i + 1) * 512],
                                    k1T_sb[p][e * 64:(e + 1) * 64,
                                              kt * 128:(kt + 1) * 128],
                                    qT_sb[p][e * 64:(e + 1) * 64,
                                             qc * 512:(qc + 1) * 512],
                                    start=True, stop=True,
                                    tile_position=(e * 64, 0))
                            nc.scalar.activation(
                                P[:, half * 1024:(half + 1) * 1024],
                                pssh[half][:], EXP, scale=SCALE)
                        if kt >= PV_LAG:
                            pv_emit(qc, kt - PV_LAG, Ps.pop(kt - PV_LAG))
                        for fn in post.get(kt, ()):
                            fn()
                    if flush:
                        for kt in range(max(0, KT - PV_LAG), KT):
                            pv_emit(qc, kt, Ps.pop(kt))
                    return Ps

                # qc0 with the kv2 pool (remaining KV slices in background)
                with tc.tile_pool(name="kv2", bufs=1, space="PSUM") as kps:
                    kvk = kps.tile([128, 512], F32, name="kvk", tag="kvk")
                    qc_loop(0, (kvk[:, :], kvk[:, 0:256]), None)

                with tc.tile_pool(name="attn2", bufs=1, space="PSUM") as a2:
                    psp = a2.tile([128, 512], F32, name="psp", tag="psp")
                    Ps = {}
                    for qc in range(1, QC):
                        Ps = qc_loop(qc, None, (trans_ps, psp),
                                     flush=(qc != QC - 1))
                    if QC == 1:
                        Ps = {}
                    # tail: last qc's PV flush + normalize + projection,
                    # per q-subtile so the chains overlap the flush. The
                    # pssh banks are free after the last exp — rotate the
                    # proj psum across 5 banks and split copies across
                    # engines so nothing serializes.
                    tail_ps = [psp[:, :],
                               pssh[0][:, 0:512], pssh[0][:, 512:1024],
                               pssh[1][:, 0:512], pssh[1][:, 512:1024]]
                    ti = 0
                    fl = range(max(0, KT - PV_LAG), KT) if QC > 1 else []
                    for qs4 in range(4):
                        for kt in fl:
                            pv_emit(QC - 1, kt, Ps[kt], qs4s=(qs4,))
                        post_chunk(QC - 1, qs4, trans_ps)
                        for dc in range(2):
                            eng = nc.vector if dc == 0 else nc.gpsimd
                            proj_group(QC - 1, qs4, dc,
                                       tail_ps[ti % 5], eng)
                            ti += 1

    nc.compile()
    return nc


# ---------------- host-side runner (inlined, self-contained) ----------------
class _Runner:
    def __init__(self, nc, n_cores):
        import jax
        from jax.sharding import Mesh, PartitionSpec
        from jax.experimental.shard_map import shard_map
        from concourse.bass2jax import (
            _bass_exec_p, install_neuronx_cc_hook, partition_id_tensor)
        install_neuronx_cc_hook()
        self.jax = jax
        self.n_cores = n_cores
        partition_name = (nc.partition_id_tensor.name
                          if nc.partition_id_tensor else None)
        in_names, out_names, out_avals, zero_outs = [], [], [], []
        for alloc in nc.m.functions[0].allocations:
            if not isinstance(alloc, mybir.MemoryLocationSet):
                continue
            name = alloc.memorylocations[0].name
            if alloc.kind == "ExternalInput":
                if name != partition_name:
                    in_names.append(name)
            elif alloc.kind == "ExternalOutput":
                shape = tuple(alloc.tensor_shape)
                dtype = mybir.dt.np(alloc.dtype)
                out_names.append(name)
                out_avals.append(jax.core.ShapedArray(shape, dtype))
                zero_outs.append(np.zeros(shape, dtype))
        self.in_names, self.out_names = in_names, out_names
        self.out_avals, self.zero_outs = out_avals, zero_outs
        n_params = len(in_names)
        self.n_params = n_params
        all_in = list(in_names) + list(out_names)
        if partition_name is not None:
            all_in.append(partition_name)

        def _body(*args):
            operands = list(args)
            if partition_name is not None:
                operands.append(partition_id_tensor())
            return tuple(_bass_exec_p.bind(
                *operands, out_avals=tuple(out_avals),
                in_names=tuple(all_in), out_names=tuple(out_names),
                lowering_input_output_aliases=(),
                sim_require_finite=True, sim_require_nnan=True, nc=nc))

        devices = jax.devices()[:n_cores]
        self.mesh = Mesh(np.asarray(devices), ("core",))
        n_outs = len(out_names)
        self.fn = jax.jit(
            shard_map(_body, mesh=self.mesh,
                      in_specs=(PartitionSpec("core"),) * (n_params + n_outs),
                      out_specs=(PartitionSpec("core"),) * n_outs,
                      check_rep=False),
            keep_unused=True)
        self._dev_args = None

    def stage_inputs(self, in_maps):
        from jax.sharding import NamedSharding, PartitionSpec
        per_core = [[np.asarray(m[n]) for n in self.in_names] for m in in_maps]
        concat_in = [
            np.concatenate([per_core[c][i] for c in range(self.n_cores)],
                           axis=0) for i in range(self.n_params)]
        concat_zeros = [np.zeros((self.n_cores * z.shape[0], *z.shape[1:]),
                                 z.dtype) for z in self.zero_outs]
        sharding = NamedSharding(self.mesh, PartitionSpec("core"))
        self._dev_args = [self.jax.device_put(a, sharding)
                          for a in (*concat_in, *concat_zeros)]
        self.jax.block_until_ready(self._dev_args)

    def run(self):
        o = self.fn(*self._dev_args)
        self.jax.block_until_ready(o)
        return o

    def results(self, out_arrs):
        return [
            {n: np.asarray(out_arrs[i]).reshape(
                self.n_cores, *self.out_avals[i].shape)[c]
             for i, n in enumerate(self.out_names)}
            for c in range(self.n_cores)]


def make_in_maps(x, y, w_q, w_qkv, w_proj):
    x = np.asarray(x, np.float32)
    y = np.asarray(y, np.float32)
    w_q = np.asarray(w_q, np.float32)
    w_qkv = np.asarray(w_qkv, np.float32)
    w_proj = np.asarray(w_proj, np.float32)
    w_qq = w_qkv[0:DIM]
    wk_f = w_qkv[DIM:2 * DIM] @ w_q      # [1024, 768]
    wv_f = w_qkv[2 * DIM:3 * DIM] @ w_q
    bf16 = mybir.dt.np(BF16)
    ident = np.eye(128, dtype=np.float32)
    ones = np.ones((128, 4), bf16)
    in_maps = []
    for core in range(NCORES):
        b, g = core // 4, core % 4
        hs = slice(g * GDIM, (g + 1) * GDIM)
        in_maps.append({
            "xT": np.ascontiguousarray(x[b].T),
            "yT": np.ascontiguousarray(y[b].T),
            "wqT": np.ascontiguousarray(w_qq[hs].T),
            "wkT": np.ascontiguousarray(wk_f[hs].T),
            "wvT": np.ascontiguousarray(wv_f[hs].T),
            "wpT": np.ascontiguousarray(w_proj[:, hs].T),
            "identb": ident,
            "onesb": ones,
        })
    return in_maps


_RUNNER = None


def kernel(x, y, w_q, w_qkv, w_proj, b_proj):
    global _RUNNER
    in_maps = make_in_maps(x, y, w_q, w_qkv, w_proj)
    if _RUNNER is None:
        _RUNNER = _Runner(build_nc(), NCORES)
    _RUNNER.stage_inputs(in_maps)
    res = _RUNNER.results(_RUNNER.run())
    full = np.zeros((B, N, DIM), np.float32)
    for core in range(NCORES):
        full[core // 4] += res[core]["out"]
    full += np.asarray(b_proj, np.float32)
    return full
